# revision 10
# baseline (speedup 1.0000x reference)
"""Distributed CrossAttention (self-attention) kernel for 8 TRN2 NeuronCores.

Problem: B=2, S=2048, D=1024, H=16, DH=64, fp32.
  q/k/v = x@W.T + b; RMSNorm(q/k over full D); RoPE; SDPA; out-proj.

Sharding (token-parallel): core c -> (batch b = c//4, token block tb = c%4,
tokens 512*tb .. 512*tb+512). Each core computes q/k/v at FULL embedding
width for its 512 tokens, so RMSNorm and RoPE are fully local (no
AllReduce). One AllGather pair (k, then v) per batch group shares the
roped/normalized k and v (bf16) across the 4 cores; SDPA and the
out-projection then run fully locally for the core's 512 queries over all
2048 keys and 16 heads. The program is rank-independent (pure SPMD): the
sharding lives entirely in the host-prepared per-core inputs.

Per-core pipeline:
  1. K projection (e-major, W streamed ec-major, fp32r, RMSNorm gains
     folded into the weights on the host), local ssq via g-column matmul,
     rs_k = exp(-0.5*ln(ssq/D+eps)) on ScalarE (ln+exp share one
     activation table with the softmax exp -> zero table thrash), RoPE
     per e-chunk pipelined with the projection; rot_k streams to the
     AllGather input buffer per chunk.
  2. V projection (t-major, x-stationary, W slices resident per e-half)
     into a padded [t, 16*(64+1)] layout whose 65th column is ones (the
     softmax denominator rides the AV matmul); v streams to the AG input.
  3. AllGather k then v over the 4-core batch group while the Q side
     (projection + rs_q with 1/sqrt(DH) folded in + RoPE) proceeds on
     the compute engines.
  4. SDPA per head pair: scoresT = k_h.T @ q_h in bf16, exp on ScalarE
     straight out of PSUM, AV accumulation over 16 key chunks;
     denominator reciprocal via DVE reciprocal_approx_fast, gpsimd
     row-broadcast, divide fused into the bf16 og eviction.
  5. Local out-projection (all 16 heads local) -> yT [1024, 512]; no
     collective and no tail after the last attention tile.
"""
import numpy as np
from contextlib import ExitStack

import concourse.bass as bass
import concourse.mybir as mybir
import concourse.tile as tile
import concourse.bacc as bacc
from concourse.bass_utils import run_bass_kernel_spmd

F32 = mybir.dt.float32
F32R = mybir.dt.float32r
BF16 = mybir.dt.bfloat16
AF = mybir.ActivationFunctionType
MUL = mybir.AluOpType.mult

B, S, D, H, DH = 2, 2048, 1024, 16, 64
EPS = 1e-5
N_CORES = 8
TL = 512            # tokens per core
NEC = D // 128      # 8 e-chunks (head pairs)
NDC = D // 128      # 8 contraction chunks
NJC = S // 128      # 16 key chunks
NJL = TL // 128     # 4 key chunks per block
GROUPS = [[0, 1, 2, 3], [4, 5, 6, 7]]
LN8 = float(np.log(1.0 / np.sqrt(DH)))

TRACE = False       # test.py flips this for profiling
DEBUG = False


def _emit(nc):
    xT = nc.declare_dram_parameter("xT", [D, TL], F32, isOutput=False)
    wq = nc.declare_dram_parameter("wq", [128, NEC * D], F32, isOutput=False)
    wk = nc.declare_dram_parameter("wk", [128, NEC * D], F32, isOutput=False)
    wv = nc.declare_dram_parameter("wv", [128, 2 * NDC * 512], F32, isOutput=False)
    wo = nc.declare_dram_parameter("wo", [128, NDC * D], F32, isOutput=False)
    bq = nc.declare_dram_parameter("bq", [D, 1], F32, isOutput=False)
    bk = nc.declare_dram_parameter("bk", [D, 1], F32, isOutput=False)
    bv = nc.declare_dram_parameter("bv", [1, D], F32, isOutput=False)
    bo = nc.declare_dram_parameter("bo", [D, 1], F32, isOutput=False)
    gq = nc.declare_dram_parameter("gq", [D, 1], F32, isOutput=False)
    gk = nc.declare_dram_parameter("gk", [D, 1], F32, isOutput=False)
    ra = nc.declare_dram_parameter("rope_a", [D, TL], F32, isOutput=False)
    rb = nc.declare_dram_parameter("rope_b", [D, TL], F32, isOutput=False)
    pm = nc.declare_dram_parameter("perm", [128, 128], F32, isOutput=False)
    yT = nc.declare_dram_parameter("yT", [D, TL], F32, isOutput=True)
    if DEBUG:
        dbg_qT = nc.declare_dram_parameter("dbg_qT", [D, TL], F32, isOutput=True)
        dbg_rows = nc.declare_dram_parameter("dbg_rows", [4, TL], F32, isOutput=True)
        dbg_kf = nc.declare_dram_parameter("dbg_kf", [128, NEC * S], F32, isOutput=True)
        dbg_qr = nc.declare_dram_parameter("dbg_qr", [128, NEC * TL], F32, isOutput=True)
        dbg_vf = nc.declare_dram_parameter("dbg_vf", [128, NJC * H * 65], F32, isOutput=True)
        dbg_den = nc.declare_dram_parameter("dbg_den", [16, TL], F32, isOutput=True)
        dbg_og = nc.declare_dram_parameter("dbg_og", [128, NEC * TL], F32, isOutput=True)
        dbg_rec = nc.declare_dram_parameter("dbg_rec", [16, TL], F32, isOutput=True)
        dbg_rcb = nc.declare_dram_parameter("dbg_rcb", [64, TL], F32, isOutput=True)

    with tile.TileContext(nc) as tc, ExitStack() as ctx:
        # ---------------- persistent pools ----------------
        kf_pool = ctx.enter_context(tc.tile_pool(name="kf", bufs=1))
        vf_pool = ctx.enter_context(tc.tile_pool(name="vf", bufs=1))
        qrot_pool = ctx.enter_context(tc.tile_pool(name="qrot", bufs=1))
        og_pool = ctx.enter_context(tc.tile_pool(name="og", bufs=1))
        stg_pool = ctx.enter_context(tc.tile_pool(name="stg", bufs=1))
        small = ctx.enter_context(tc.tile_pool(name="small", bufs=1))
        dram = ctx.enter_context(tc.tile_pool(name="dram", bufs=1, space="DRAM"))

        # ---------------- constants / small loads ----------------
        bq_sb, bk_sb, bo_sb, gq_sb, gk_sb = [], [], [], [], []
        for ec in range(NEC):
            sl = slice(128 * ec, 128 * ec + 128)
            t = small.tile([128, 1], F32, tag=f"bq{ec}", name=f"bq{ec}")
            nc.sync.dma_start(t[:], bq[sl, :]); bq_sb.append(t)
            t = small.tile([128, 1], F32, tag=f"bk{ec}", name=f"bk{ec}")
            nc.sync.dma_start(t[:], bk[sl, :]); bk_sb.append(t)
            t = small.tile([128, 1], F32, tag=f"bo{ec}", name=f"bo{ec}")
            nc.sync.dma_start(t[:], bo[sl, :]); bo_sb.append(t)
            t = small.tile([128, 1], F32R, tag=f"gq{ec}", name=f"gq{ec}")
            nc.sync.dma_start(t[:], gq[sl, :].bitcast(F32R)); gq_sb.append(t)
            t = small.tile([128, 1], F32R, tag=f"gk{ec}", name=f"gk{ec}")
            nc.sync.dma_start(t[:], gk[sl, :].bitcast(F32R)); gk_sb.append(t)
        bv_sb = small.tile([1, D], F32R, tag="bvrow")
        nc.sync.dma_start(bv_sb[:], bv[:].bitcast(F32R))
        ones1 = small.tile([1, 128], F32, tag="ones1")
        nc.vector.memset(ones1[:], 1.0)
        ones1r = small.tile([1, 128], F32R, tag="ones1r")
        nc.vector.tensor_copy(ones1r[:], ones1[:])
        ones16 = small.tile([128, 16], F32, tag="ones16")
        nc.vector.memset(ones16[:], 1.0)
        eps_t = small.tile([1, 1], F32, tag="eps")
        nc.vector.memset(eps_t[:], EPS)
        ln8_t = small.tile([1, 1], F32, tag="ln8")
        nc.vector.memset(ln8_t[:], LN8)
        perm_sb = small.tile([128, 128], F32R, tag="perm")
        nc.sync.dma_start(perm_sb[:], pm[:].bitcast(F32R))

        # persistent activation storage
        # k_full: e-major [128, S] per e-chunk (head pair), slot r = block r
        kf = kf_pool.tile([128, NEC * S], BF16, tag="kf", name="kf", bufs=1)
        kf_c = [kf[:, S * ec:S * ec + S] for ec in range(NEC)]
        # v_full: t-major [128, 16*65] per 128-token chunk
        vf = vf_pool.tile([128, NJC * H * 65], BF16, tag="vf", name="vf", bufs=1)
        vf_c = [vf[:, H * 65 * jt:H * 65 * (jt + 1)] for jt in range(NJC)]
        # q_rot: e-major [128, TL] per e-chunk
        qrot = qrot_pool.tile([128, NEC * TL], BF16, tag="qrot", name="qrot", bufs=1)
        qrot_c = [qrot[:, TL * ec:TL * ec + TL] for ec in range(NEC)]
        # attention output (divided), e-major per head pair
        og = og_pool.tile([128, NEC * TL], BF16, tag="og", name="og", bufs=1)
        og_c = [og[:, TL * ec:TL * ec + TL] for ec in range(NEC)]
        # own-block staging for the AG inputs
        kst = stg_pool.tile([128, NEC * TL], BF16, tag="kst", name="kst", bufs=1)
        kst_c = [kst[:, TL * ec:TL * ec + TL] for ec in range(NEC)]
        vst = stg_pool.tile([128, NJL * H * 65], BF16, tag="vst", name="vst", bufs=1)
        vst_c = [vst[:, H * 65 * ts:H * 65 * (ts + 1)] for ts in range(NJL)]

        # AllGather DRAM buffers
        ag_k_in = dram.tile([D, TL], BF16, tag="agki", name="ag_k_in")
        ag_k_out = dram.tile([4 * D, TL], BF16, tag="agko", name="ag_k_out")
        ag_v_in = dram.tile([NJL * 128, H * 65], BF16, tag="agvi", name="ag_v_in")
        ag_v_out = dram.tile([NJC * 128, H * 65], BF16, tag="agvo", name="ag_v_out")

        # ---------------- phase A ----------------
        with ExitStack() as actx:
            x_pool = actx.enter_context(tc.tile_pool(name="xp", bufs=8))
            w_pool = actx.enter_context(tc.tile_pool(name="wp", bufs=3))
            wv_pool = actx.enter_context(tc.tile_pool(name="wvp", bufs=8))
            qk_pool = actx.enter_context(tc.tile_pool(name="qk", bufs=8))
            sq_pool = actx.enter_context(tc.tile_pool(name="sq", bufs=2))
            ab_pool = actx.enter_context(tc.tile_pool(name="ab", bufs=4))
            tmp_pool = actx.enter_context(tc.tile_pool(name="tmp", bufs=6))
            rs_pool = actx.enter_context(tc.tile_pool(name="rs", bufs=4))
            pproj = actx.enter_context(tc.tile_pool(name="pproj", bufs=2, space="PSUM"))
            pv = actx.enter_context(tc.tile_pool(name="pv", bufs=2, space="PSUM"))
            pssq = actx.enter_context(tc.tile_pool(name="pssq", bufs=1, space="PSUM"))
            psw = actx.enter_context(tc.tile_pool(name="psw", bufs=2, space="PSUM"))

            # x tiles, loaded once, resident for K/V/Q projections
            xt = [x_pool.tile([128, TL], F32R, tag="xt", name=f"xt{i}")
                  for i in range(NDC)]
            for dc in range(NDC):
                nc.sync.dma_start(xt[dc][:], xT[128 * dc:128 * dc + 128, :].bitcast(F32R))

            def proj_pass(wparam, bias_sb, g_sb, kind):
                """e-major projection (K or Q): per ec, stream W slice, 8
                matmuls, evict with bias, square for ssq."""
                dst = []
                ssq_ps = pssq.tile([1, TL], F32, tag="pssq", name=f"ssq_{kind}")
                for ec in range(NEC):
                    wsl = w_pool.tile([128, D], F32R, tag="w", name=f"w_{kind}{ec}")
                    nc.sync.dma_start(wsl[:], wparam[:, D * ec:D * ec + D].bitcast(F32R))
                    qp = pproj.tile([128, TL], F32, tag="pproj", name=f"p_{kind}{ec}")
                    for dc in range(NDC):
                        nc.tensor.matmul(
                            qp[:], wsl[:, 128 * dc:128 * dc + 128], xt[dc][:],
                            start=(dc == 0), stop=(dc == NDC - 1),
                        )
                    d = qk_pool.tile([128, TL], F32R, tag="qk", name=f"{kind}T{ec}")
                    nc.scalar.activation(d[:], qp[:], AF.Identity, bias=bias_sb[ec][:])
                    dst.append(d)
                    sq = sq_pool.tile([128, TL], F32R, tag="sq", name=f"sq_{kind}{ec}")
                    nc.scalar.activation(sq[:], qp[:], AF.Square, bias=bias_sb[ec][:])
                    nc.tensor.matmul(
                        ssq_ps[:], g_sb[ec][:], sq[:],
                        start=(ec == 0), stop=(ec == NEC - 1),
                    )
                ssq_c = rs_pool.tile([1, TL], F32, tag="rs", name=f"ssqc_{kind}")
                nc.vector.tensor_copy(ssq_c[:], ssq_ps[:])
                return dst, ssq_c

            def rs_row(ssq_c, kind, fold8):
                """rs = exp(-0.5*ln(ssq/D + eps) [+ ln(1/8)]) broadcast to
                [128, TL]; ln+exp live in the same activation table as the
                softmax exp."""
                lnv = rs_pool.tile([1, TL], F32, tag="rs", name=f"ln_{kind}")
                nc.scalar.activation(lnv[:], ssq_c[:], AF.Ln, scale=1.0 / D,
                                     bias=eps_t[:])
                rsr = rs_pool.tile([1, TL], F32, tag="rs", name=f"rsr_{kind}")
                if fold8:
                    nc.scalar.activation(rsr[:], lnv[:], AF.Exp, scale=-0.5,
                                         bias=ln8_t[:])
                else:
                    nc.scalar.activation(rsr[:], lnv[:], AF.Exp, scale=-0.5)
                rsb = rs_pool.tile([128, TL], F32, tag="rsb", name=f"rsb_{kind}",
                                   bufs=2)
                nc.gpsimd.partition_broadcast(rsb[:], rsr[0:1, :], channels=128)
                return rsb

            def rope_chunk(src, rsb, dst_ap, ec, kind):
                """dst = (A*src + B*(P@src)) * rs for one e-chunk."""
                esl = slice(128 * ec, 128 * ec + 128)
                at = ab_pool.tile([128, TL], F32, tag="ab", name=f"at_{kind}{ec}")
                bt = ab_pool.tile([128, TL], F32, tag="ab", name=f"bt_{kind}{ec}")
                nc.sync.dma_start(at[:], ra[esl, :])
                nc.sync.dma_start(bt[:], rb[esl, :])
                swp = psw.tile([128, TL], F32, tag="psw", name=f"sw_{kind}{ec}")
                nc.tensor.matmul(swp[:], perm_sb[:], src[:], start=True, stop=True)
                t1 = tmp_pool.tile([128, TL], F32, tag="tmp", name=f"t1_{kind}{ec}")
                nc.vector.tensor_tensor(t1[:], at[:], src[:], MUL)
                t2 = tmp_pool.tile([128, TL], F32, tag="tmp", name=f"t2_{kind}{ec}")
                nc.vector.tensor_tensor(t2[:], bt[:], swp[:], MUL)
                t3 = tmp_pool.tile([128, TL], F32, tag="tmp", name=f"t3_{kind}{ec}")
                nc.vector.tensor_add(t3[:], t1[:], t2[:])
                nc.vector.tensor_tensor(dst_ap, t3[:], rsb[:], MUL)

            # ---- K: projection + rs + rope, stream to AG input ----
            kT, ssq_k = proj_pass(wk, bk_sb, gk_sb, "k")
            rsb_k = rs_row(ssq_k, "k", fold8=False)
            ssq_k_dbg, rsb_k_dbg = ssq_k, rsb_k
            for ec in range(NEC):
                rope_chunk(kT[ec], rsb_k, kst_c[ec][:], ec, "k")
                nc.sync.dma_start(ag_k_in[128 * ec:128 * ec + 128, :], kst_c[ec][:])

            # ---- V: t-major projection into padded head-block layout ----
            for ts in range(NJL):
                ones_cols = vst_c[ts].rearrange("p (h c) -> p h c", c=65)[:, :, 64:65]
                nc.vector.tensor_copy(
                    ones_cols, ones16[:].rearrange("p (f o) -> p f o", o=1)
                )
            for eh in range(2):
                wvt = [wv_pool.tile([128, 512], F32R, tag="wv", name=f"wv{eh}_{i}")
                       for i in range(NDC)]
                for dc in range(NDC):
                    nc.sync.dma_start(
                        wvt[dc][:],
                        wv[:, 512 * (2 * dc + eh):512 * (2 * dc + eh) + 512]
                        .bitcast(F32R),
                    )
                for ts in range(NJL):
                    vsl = slice(128 * ts, 128 * ts + 128)
                    vp = pv.tile([128, 512], F32, tag="pv", name=f"pv{eh}{ts}")
                    for dc in range(NDC):
                        nc.tensor.matmul(
                            vp[:], xt[dc][:, vsl], wvt[dc][:],
                            start=(dc == 0), stop=False,
                        )
                    nc.tensor.matmul(
                        vp[:], ones1r[:], bv_sb[:, 512 * eh:512 * eh + 512],
                        start=False, stop=True,
                    )
                    nc.scalar.activation(
                        vst_c[ts].rearrange("p (h c) -> p h c", c=65)[
                            :, 8 * eh:8 * eh + 8, 0:64],
                        vp[:].rearrange("p (h c) -> p h c", c=64),
                        AF.Identity,
                    )
            for ts in range(NJL):
                nc.sync.dma_start(ag_v_in[128 * ts:128 * ts + 128, :], vst_c[ts])

            # ---- AllGather k then v over the batch group ----
            nc.gpsimd.collective_compute(
                "AllGather", mybir.AluOpType.bypass, replica_groups=GROUPS,
                ins=[ag_k_in[:].opt()], outs=[ag_k_out[:].opt()],
            )
            nc.gpsimd.collective_compute(
                "AllGather", mybir.AluOpType.bypass, replica_groups=GROUPS,
                ins=[ag_v_in[:].opt()], outs=[ag_v_out[:].opt()],
            )
            # read back all four blocks into slot r = block r
            for r in range(4):
                nc.sync.dma_start(
                    kf[:].rearrange("p (ec r t) -> p ec r t", ec=NEC, r=4)[:, :, r, :],
                    ag_k_out[D * r:D * r + D, :]
                    .rearrange("(ec p) t -> p ec t", p=128),
                )
                nc.sync.dma_start(
                    vf[:].rearrange("p (jt f) -> p jt f", jt=NJC)[
                        :, NJL * r:NJL * r + NJL, :],
                    ag_v_out[NJL * 128 * r:NJL * 128 * (r + 1), :]
                    .rearrange("(ts p) f -> p ts f", p=128),
                )

            # ---- Q: projection + rs (with 1/sqrt(DH) folded) + rope ----
            qT, ssq_q = proj_pass(wq, bq_sb, gq_sb, "q")
            rsb_q = rs_row(ssq_q, "q", fold8=True)
            for ec in range(NEC):
                rope_chunk(qT[ec], rsb_q, qrot_c[ec][:], ec, "q")
            if DEBUG:
                for ec in range(NEC):
                    nc.sync.dma_start(dbg_qT[128 * ec:128 * ec + 128, :],
                                      qT[ec][:].bitcast(F32))
                nc.sync.dma_start(dbg_rows[0:1, :], ssq_q[:])
                nc.sync.dma_start(dbg_rows[1:2, :], ssq_k_dbg[:])
                nc.sync.dma_start(dbg_rows[2:3, :], rsb_q[0:1, :])
                nc.sync.dma_start(dbg_rows[3:4, :], rsb_k_dbg[0:1, :])
                dbgf = qk_pool.tile([128, TL], F32, tag="qk", name="dbgf")
                for ec in range(NEC):
                    nc.vector.tensor_copy(dbgf[:], qrot_c[ec][:])
                    nc.sync.dma_start(
                        dbg_qr[:, TL * ec:TL * ec + TL], dbgf[:])
                for ec in range(NEC):
                    for r in range(4):
                        nc.vector.tensor_copy(dbgf[:], kf_c[ec][:, TL * r:TL * r + TL])
                        nc.sync.dma_start(
                            dbg_kf[:, S * ec + TL * r:S * ec + TL * r + TL], dbgf[:])
                for jt in range(NJC):
                    dvt = sq_pool.tile([128, H * 65], F32, tag="dvt", name="dvt",
                                       bufs=2)
                    nc.vector.tensor_copy(dvt[:], vf_c[jt])
                    nc.sync.dma_start(
                        dbg_vf[:, H * 65 * jt:H * 65 * (jt + 1)], dvt[:])

        # ---------------- phase B: attention ----------------
        with ExitStack() as bctx:
            exp_pool = bctx.enter_context(tc.tile_pool(name="exp", bufs=12))
            dv_pool = bctx.enter_context(tc.tile_pool(name="dv", bufs=4))
            rec_pool = bctx.enter_context(tc.tile_pool(name="rec", bufs=4))
            psc = bctx.enter_context(tc.tile_pool(name="psc", bufs=2, space="PSUM"))
            pav = bctx.enter_context(tc.tile_pool(name="pav", bufs=4, space="PSUM"))

            for hp in range(NEC):
                av = [pav.tile([65, TL], F32, tag="pav", name=f"av{hp}_{i}")
                      for i in range(2)]
                for jc in range(NJC):
                    jsl = slice(128 * jc, 128 * jc + 128)
                    sc = psc.tile([128, 2 * TL], F32, tag="psc", name=f"sc{hp}_{jc}")
                    for hh in range(2):
                        psl = slice(64 * hh, 64 * hh + 64)
                        nc.tensor.matmul(
                            sc[:, TL * hh:TL * hh + TL],
                            kf_c[hp][psl, jsl], qrot_c[hp][psl, :],
                            start=True, stop=True,
                        )
                    ex = exp_pool.tile([128, 2 * TL], BF16, tag="exp",
                                       name=f"ex{hp}_{jc}")
                    nc.scalar.activation(ex[:], sc[:], AF.Exp)
                    for hh in range(2):
                        h4 = 2 * hp + hh
                        nc.tensor.matmul(
                            av[hh][:],
                            vf_c[jc][:, 65 * h4:65 * h4 + 65],
                            ex[:, TL * hh:TL * hh + TL],
                            start=(jc == 0), stop=(jc == NJC - 1),
                        )
                for hh in range(2):
                    avs = dv_pool.tile([65, TL], F32, tag="avs", name=f"avs{hp}{hh}")
                    nc.vector.tensor_copy(avs[:], av[hh][:])
                    rin = rec_pool.tile([1, TL], F32, tag="rin", name=f"rin{hp}{hh}")
                    nc.vector.tensor_copy(rin[:], avs[64:65, :])
                    rec = rec_pool.tile([1, TL], F32, tag="rec", name=f"rec{hp}{hh}")
                    nc.vector.reciprocal_approx_fast(rec[:], rin[:])
                    if DEBUG:
                        nc.sync.dma_start(
                            dbg_den[2 * hp + hh:2 * hp + hh + 1, :], avs[64:65, :])
                    rcb = rec_pool.tile([64, TL], F32, tag="rcb", name=f"rcb{hp}{hh}")
                    nc.gpsimd.partition_broadcast(rcb[:], rec[0:1, :], channels=64)
                    if DEBUG:
                        nc.sync.dma_start(
                            dbg_rec[2 * hp + hh:2 * hp + hh + 1, :], rec[:])
                        if hp == 0 and hh == 0:
                            nc.sync.dma_start(dbg_rcb[:], rcb[:])
                    nc.vector.tensor_tensor(
                        og_c[hp][64 * hh:64 * hh + 64, :], avs[0:64, :], rcb[:], MUL
                    )

        if DEBUG:
            with ExitStack() as dctx:
                dpool = dctx.enter_context(tc.tile_pool(name="dbgog", bufs=2))
                for ec in range(NEC):
                    dt_ = dpool.tile([128, TL], F32, tag="do", name=f"do{ec}")
                    nc.vector.tensor_copy(dt_[:], og_c[ec][:])
                    nc.sync.dma_start(dbg_og[:, TL * ec:TL * ec + TL], dt_[:])

        # ---------------- out-projection (fully local) ----------------
        with ExitStack() as octx:
            wo_pool = octx.enter_context(tc.tile_pool(name="wop", bufs=3))
            y_pool = octx.enter_context(tc.tile_pool(name="y", bufs=2))
            py = octx.enter_context(tc.tile_pool(name="py", bufs=2, space="PSUM"))
            for dco in range(NDC):
                wos = wo_pool.tile([128, D], BF16, tag="wo", name=f"wo{dco}")
                nc.gpsimd.dma_start(wos[:], wo[:, D * dco:D * dco + D])
                yp = py.tile([128, TL], F32, tag="py", name=f"yp{dco}")
                for ec in range(NEC):
                    nc.tensor.matmul(
                        yp[:], wos[:, 128 * ec:128 * ec + 128], og_c[ec][:],
                        start=(ec == 0), stop=(ec == NEC - 1),
                    )
                ys = y_pool.tile([128, TL], F32, tag="y", name=f"ys{dco}")
                nc.scalar.activation(ys[:], yp[:], AF.Identity, bias=bo_sb[dco][:])
                nc.sync.dma_start(yT[128 * dco:128 * dco + 128, :], ys[:])


def build_nc():
    nc = bacc.Bacc("TRN2", target_bir_lowering=False, debug=False,
                   num_devices=N_CORES)
    _emit(nc)
    nc.compile()
    return nc


_NC_CACHE = None


def _get_nc():
    global _NC_CACHE
    if _NC_CACHE is None:
        _NC_CACHE = build_nc()
    return _NC_CACHE


def _host_prep(inputs):
    x = np.ascontiguousarray(np.asarray(inputs["x"], dtype=np.float32))
    pe = np.asarray(inputs["pe"], dtype=np.float32)[0, 0]      # [S, D//2, 2, 2]
    Wq = np.asarray(inputs["Wq"], dtype=np.float32)
    bq = np.asarray(inputs["bq"], dtype=np.float32)
    Wk = np.asarray(inputs["Wk"], dtype=np.float32)
    bk = np.asarray(inputs["bk"], dtype=np.float32)
    Wv = np.asarray(inputs["Wv"], dtype=np.float32)
    bv = np.asarray(inputs["bv"], dtype=np.float32)
    qn = np.asarray(inputs["qn_w"], dtype=np.float32)
    kn = np.asarray(inputs["kn_w"], dtype=np.float32)
    Wo = np.asarray(inputs["Wo"], dtype=np.float32)
    bo = np.asarray(inputs["bo"], dtype=np.float32)

    Wq_f = Wq * qn[:, None]
    bq_f = qn * bq
    Wk_f = Wk * kn[:, None]
    bk_f = kn * bk

    A = np.empty((D, S), np.float32)
    Bm = np.empty((D, S), np.float32)
    A[0::2, :] = pe[:, :, 0, 0].T
    A[1::2, :] = pe[:, :, 1, 1].T
    Bm[0::2, :] = pe[:, :, 0, 1].T
    Bm[1::2, :] = pe[:, :, 1, 0].T

    perm = np.zeros((128, 128), np.float32)
    idx = np.arange(64)
    perm[2 * idx, 2 * idx + 1] = 1.0
    perm[2 * idx + 1, 2 * idx] = 1.0

    def pack_ec(m):
        # m: [D_in, D_out] -> [128, NEC * D]; slice ec holds, for output
        # chunk ec, the per-dc stationary blocks side by side:
        # out[p, ec*D + dc*128 + j] = m[dc*128 + p, ec*128 + j]
        return np.ascontiguousarray(
            m.reshape(NDC, 128, NEC, 128).transpose(1, 2, 0, 3).reshape(128, NEC * D)
        )

    def pack_v(m):
        # m: [D_in, D_out] -> [128, 2*NDC*512]; slice (dc, eh) at
        # 512*(2*dc+eh) holds the moving block m[dc*128:+128, eh*512:+512]
        return np.ascontiguousarray(
            m.reshape(NDC, 128, 2, 512).transpose(1, 0, 2, 3).reshape(128, 2 * NDC * 512)
        )

    def pack_o(m):
        # m: [D_e, D_out] -> [128, NDC * D]; slice dco holds, for output
        # chunk dco, the per-ec stationary blocks side by side:
        # out[p, dco*D + ec*128 + j] = m[ec*128 + p, dco*128 + j]
        return np.ascontiguousarray(
            m.reshape(NEC, 128, NDC, 128).transpose(1, 2, 0, 3).reshape(128, NDC * D)
        )

    wq_p = pack_ec(Wq_f.T)
    wk_p = pack_ec(Wk_f.T)
    wv_p = pack_v(Wv.T)
    wo_p = pack_o(Wo.T)
    gq_c = np.ascontiguousarray(1.0 / qn ** 2)[:, None]
    gk_c = np.ascontiguousarray(1.0 / kn ** 2)[:, None]
    bq_c = np.ascontiguousarray(bq_f)[:, None]
    bk_c = np.ascontiguousarray(bk_f)[:, None]
    bv_c = np.ascontiguousarray(bv)[None, :]
    bo_c = np.ascontiguousarray(bo)[:, None]

    xTs = [np.ascontiguousarray(x[b].T) for b in range(B)]
    in_maps = []
    for c in range(N_CORES):
        b, tb = c // 4, c % 4
        tsl = slice(TL * tb, TL * tb + TL)
        in_maps.append({
            "xT": np.ascontiguousarray(xTs[b][:, tsl]),
            "wq": wq_p,
            "wk": wk_p,
            "wv": wv_p,
            "wo": wo_p,
            "bq": bq_c,
            "bk": bk_c,
            "bv": bv_c,
            "bo": bo_c,
            "gq": gq_c,
            "gk": gk_c,
            "rope_a": np.ascontiguousarray(A[:, tsl]),
            "rope_b": np.ascontiguousarray(Bm[:, tsl]),
            "perm": perm,
        })
    return in_maps


def kernel(**inputs):
    nc = _get_nc()
    in_maps = _host_prep(inputs)
    res = run_bass_kernel_spmd(
        nc, in_maps, core_ids=list(range(N_CORES)), trace=TRACE
    )
    if TRACE and res.exec_time_ns is not None:
        print(f"HW exec time: {res.exec_time_ns} ns")
    y = np.empty((B, S, D), np.float32)
    for c in range(N_CORES):
        b, tb = c // 4, c % 4
        y[b][TL * tb:TL * tb + TL, :] = res.results[c]["yT"].T
    return y


# revision 11
# speedup vs baseline: 1.1120x; 1.1120x over previous
"""Distributed CrossAttention (self-attention) kernel for 8 TRN2 NeuronCores.

Problem: B=2, S=2048, D=1024, H=16, DH=64, fp32.
  q/k/v = x@W.T + b; RMSNorm(q/k over full D); RoPE; SDPA; out-proj.

Sharding (token-parallel): core c -> (batch b = c//4, token block tb = c%4,
tokens 512*tb .. 512*tb+512). Each core computes q/k at FULL embedding
width for its 512 tokens, so RMSNorm and RoPE are fully local (no
AllReduce). One AllGather per batch group shares the roped/normalized k
(bf16); v is cheap to project, so every core computes the FULL v locally
from a bf16 copy of x (replicated flops instead of a second collective).
SDPA and the out-projection then run fully locally for the core's 512
queries over all 2048 keys and 16 heads. The program is rank-independent
(pure SPMD): the sharding lives entirely in the host-prepared inputs.

Per-core pipeline:
  1. K projection (e-major, W streamed ec-major, fp32r, RMSNorm gains
     folded into the weights on the host), local ssq via g-column matmul,
     rs_k = exp(-0.5*ln(ssq/D+eps)) on ScalarE (ln+exp share one
     activation table with the softmax exp -> zero table thrash), RoPE
     per e-chunk pipelined with the projection; rot_k streams to the
     AllGather input buffer per chunk, AG fires early.
  2. Q projection + rs_q (1/sqrt(DH) folded via the exp bias) + RoPE
     while the AG is in flight.
  3. Full-v projection (t-major, bf16, x-stationary from a jt-major bf16
     x copy) into a padded [t, 16*(64+1)] layout whose 65th column is
     ones (the softmax denominator rides the AV matmul).
  4. SDPA per head pair: scoresT = k_h.T @ q_h in bf16, exp on ScalarE
     straight out of PSUM [128,1024], AV accumulation over 16 key
     chunks; denominator reciprocal via DVE reciprocal_approx_fast,
     gpsimd row-broadcast, divide fused into the bf16 og eviction.
  5. Local out-projection (wo prefetched in bf16 at kernel start) ->
     yT [1024, 512]; no collective and no tail stall.
"""
import numpy as np
from contextlib import ExitStack

import concourse.bass as bass
import concourse.mybir as mybir
import concourse.tile as tile
import concourse.bacc as bacc
from concourse.bass_utils import run_bass_kernel_spmd

F32 = mybir.dt.float32
F32R = mybir.dt.float32r
BF16 = mybir.dt.bfloat16
AF = mybir.ActivationFunctionType
MUL = mybir.AluOpType.mult

B, S, D, H, DH = 2, 2048, 1024, 16, 64
EPS = 1e-5
N_CORES = 8
TL = 512            # tokens per core
NEC = D // 128      # 8 e-chunks (head pairs)
NDC = D // 128      # 8 contraction chunks
NJC = S // 128      # 16 key chunks
GROUPS = [[0, 1, 2, 3], [4, 5, 6, 7]]
LN8 = float(np.log(1.0 / np.sqrt(DH)))

TRACE = False       # test.py flips this for profiling


def _emit(nc):
    xL = nc.declare_dram_parameter("xL", [128, NDC * TL], F32, isOutput=False)
    xf = nc.declare_dram_parameter("xf", [128, NJC * D], BF16, isOutput=False)
    wq = nc.declare_dram_parameter("wq", [128, NEC * D], F32, isOutput=False)
    wk = nc.declare_dram_parameter("wk", [128, NEC * D], F32, isOutput=False)
    wv = nc.declare_dram_parameter("wv", [128, 2 * NDC * 512], BF16, isOutput=False)
    wo = nc.declare_dram_parameter("wo", [128, NDC * D], F32, isOutput=False)
    # smalls: cols 0-7 bq, 8-15 bk, 16-23 bo, [0,24] eps, [0,25] ln(1/8)
    smalls = nc.declare_dram_parameter("smalls", [128, 26], F32, isOutput=False)
    gqk = nc.declare_dram_parameter("gqk", [128, 16], F32, isOutput=False)
    bv = nc.declare_dram_parameter("bv", [1, D], BF16, isOutput=False)
    ra = nc.declare_dram_parameter("rope_a", [D, TL], F32, isOutput=False)
    rb = nc.declare_dram_parameter("rope_b", [D, TL], F32, isOutput=False)
    pm = nc.declare_dram_parameter("perm", [128, 128], F32, isOutput=False)
    yT = nc.declare_dram_parameter("yT", [D, TL], F32, isOutput=True)

    with tile.TileContext(nc) as tc, ExitStack() as ctx:
        # ---------------- persistent pools ----------------
        kf_pool = ctx.enter_context(tc.tile_pool(name="kf", bufs=1))
        vf_pool = ctx.enter_context(tc.tile_pool(name="vf", bufs=1))
        qrot_pool = ctx.enter_context(tc.tile_pool(name="qrot", bufs=1))
        og_pool = ctx.enter_context(tc.tile_pool(name="og", bufs=1))
        wo_pool = ctx.enter_context(tc.tile_pool(name="wop", bufs=1))
        small = ctx.enter_context(tc.tile_pool(name="small", bufs=1))
        dram = ctx.enter_context(tc.tile_pool(name="dram", bufs=1, space="DRAM"))

        # ---------------- constants / small loads ----------------
        sm = small.tile([128, 26], F32, tag="sm", name="sm")
        nc.sync.dma_start(sm[:], smalls[:])
        gq_t = small.tile([128, 16], F32R, tag="gqk", name="gq_t")
        nc.sync.dma_start(gq_t[:], gqk[:].bitcast(F32R))
        bq_sb = [sm[:, ec:ec + 1] for ec in range(NEC)]
        bk_sb = [sm[:, 8 + ec:9 + ec] for ec in range(NEC)]
        bo_sb = [sm[:, 16 + ec:17 + ec] for ec in range(NEC)]
        eps_t = sm[0:1, 24:25]
        ln8_t = sm[0:1, 25:26]
        gq_sb = [gq_t[:, ec:ec + 1] for ec in range(NEC)]
        gk_sb = [gq_t[:, 8 + ec:9 + ec] for ec in range(NEC)]
        bv_sb = small.tile([1, D], BF16, tag="bvrow")
        nc.sync.dma_start(bv_sb[:], bv[:])
        ones1b = small.tile([1, 128], BF16, tag="ones1b")
        nc.vector.memset(ones1b[:], 1.0)
        ones16 = small.tile([128, 16], F32, tag="ones16")
        nc.vector.memset(ones16[:], 1.0)
        perm_sb = small.tile([128, 128], F32R, tag="perm")
        nc.sync.dma_start(perm_sb[:], pm[:].bitcast(F32R))

        # wo prefetch (bf16 convert on the gpsimd SWDGE queue)
        wo_sb = wo_pool.tile([128, NDC * D], BF16, tag="wo", name="wo_sb", bufs=1)
        for dco in range(NDC):
            nc.gpsimd.dma_start(
                wo_sb[:, D * dco:D * dco + D], wo[:, D * dco:D * dco + D]
            )

        # persistent activation storage
        kf = kf_pool.tile([128, NEC * S], BF16, tag="kf", name="kf", bufs=1)
        kf_c = [kf[:, S * ec:S * ec + S] for ec in range(NEC)]
        vf = vf_pool.tile([128, NJC * H * 65], BF16, tag="vf", name="vf", bufs=1)
        vf_c = [vf[:, H * 65 * jt:H * 65 * (jt + 1)] for jt in range(NJC)]
        qrot = qrot_pool.tile([128, NEC * TL], BF16, tag="qrot", name="qrot", bufs=1)
        qrot_c = [qrot[:, TL * ec:TL * ec + TL] for ec in range(NEC)]
        og = og_pool.tile([128, NEC * TL], BF16, tag="og", name="og", bufs=1)
        og_c = [og[:, TL * ec:TL * ec + TL] for ec in range(NEC)]

        # AllGather DRAM buffers
        ag_k_in = dram.tile([D, TL], BF16, tag="agki", name="ag_k_in")
        ag_k_out = dram.tile([4 * D, TL], BF16, tag="agko", name="ag_k_out")

        # ---------------- phase A ----------------
        with ExitStack() as actx:
            x_pool = actx.enter_context(tc.tile_pool(name="xp", bufs=8))
            xf_pool = actx.enter_context(tc.tile_pool(name="xfp", bufs=3))
            w_pool = actx.enter_context(tc.tile_pool(name="wp", bufs=3))
            wv_pool = actx.enter_context(tc.tile_pool(name="wvp", bufs=16))
            qk_pool = actx.enter_context(tc.tile_pool(name="qk", bufs=8))
            kst_pool = actx.enter_context(tc.tile_pool(name="kst", bufs=2))
            sq_pool = actx.enter_context(tc.tile_pool(name="sq", bufs=2))
            ab_pool = actx.enter_context(tc.tile_pool(name="ab", bufs=4))
            tmp_pool = actx.enter_context(tc.tile_pool(name="tmp", bufs=6))
            rs_pool = actx.enter_context(tc.tile_pool(name="rs", bufs=3))
            pproj = actx.enter_context(tc.tile_pool(name="pproj", bufs=2, space="PSUM"))
            pv = actx.enter_context(tc.tile_pool(name="pv", bufs=2, space="PSUM"))
            pssq = actx.enter_context(tc.tile_pool(name="pssq", bufs=1, space="PSUM"))
            psw = actx.enter_context(tc.tile_pool(name="psw", bufs=2, space="PSUM"))

            # local x (f32, for Q/K projections), loaded once
            xt = [x_pool.tile([128, TL], F32R, tag="xt", name=f"xt{i}")
                  for i in range(NDC)]
            for dc in range(NDC):
                nc.sync.dma_start(
                    xt[dc][:], xL[:, TL * dc:TL * dc + TL].bitcast(F32R)
                )

            def proj_pass(wparam, bias_sb, g_sb, kind):
                dst = []
                ssq_ps = pssq.tile([1, TL], F32, tag="pssq", name=f"ssq_{kind}")
                for ec in range(NEC):
                    wsl = w_pool.tile([128, D], F32R, tag="w", name=f"w_{kind}{ec}")
                    nc.sync.dma_start(wsl[:], wparam[:, D * ec:D * ec + D].bitcast(F32R))
                    qp = pproj.tile([128, TL], F32, tag="pproj", name=f"p_{kind}{ec}")
                    for dc in range(NDC):
                        nc.tensor.matmul(
                            qp[:], wsl[:, 128 * dc:128 * dc + 128], xt[dc][:],
                            start=(dc == 0), stop=(dc == NDC - 1),
                        )
                    d = qk_pool.tile([128, TL], F32R, tag="qk", name=f"{kind}T{ec}")
                    nc.scalar.activation(d[:], qp[:], AF.Identity, bias=bias_sb[ec])
                    dst.append(d)
                    sq = sq_pool.tile([128, TL], F32R, tag="sq", name=f"sq_{kind}{ec}")
                    nc.scalar.activation(sq[:], qp[:], AF.Square, bias=bias_sb[ec])
                    nc.tensor.matmul(
                        ssq_ps[:], g_sb[ec], sq[:],
                        start=(ec == 0), stop=(ec == NEC - 1),
                    )
                ssq_c = rs_pool.tile([1, TL], F32, tag="rs", name=f"ssqc_{kind}")
                nc.vector.tensor_copy(ssq_c[:], ssq_ps[:])
                return dst, ssq_c

            def rs_row(ssq_c, kind, fold8):
                lnv = rs_pool.tile([1, TL], F32, tag="rs", name=f"ln_{kind}")
                nc.scalar.activation(lnv[:], ssq_c[:], AF.Ln, scale=1.0 / D,
                                     bias=eps_t)
                rsr = rs_pool.tile([1, TL], F32, tag="rs", name=f"rsr_{kind}")
                if fold8:
                    nc.scalar.activation(rsr[:], lnv[:], AF.Exp, scale=-0.5,
                                         bias=ln8_t)
                else:
                    nc.scalar.activation(rsr[:], lnv[:], AF.Exp, scale=-0.5)
                rsb = rs_pool.tile([128, TL], F32, tag="rsb", name=f"rsb_{kind}",
                                   bufs=2)
                nc.gpsimd.partition_broadcast(rsb[:], rsr[0:1, :], channels=128)
                return rsb

            def rope_chunk(src, rsb, dst_ap, ec, kind):
                esl = slice(128 * ec, 128 * ec + 128)
                at = ab_pool.tile([128, TL], F32, tag="ab", name=f"at_{kind}{ec}")
                bt = ab_pool.tile([128, TL], F32, tag="ab", name=f"bt_{kind}{ec}")
                nc.sync.dma_start(at[:], ra[esl, :])
                nc.sync.dma_start(bt[:], rb[esl, :])
                swp = psw.tile([128, TL], F32, tag="psw", name=f"sw_{kind}{ec}")
                nc.tensor.matmul(swp[:], perm_sb[:], src[:], start=True, stop=True)
                t1 = tmp_pool.tile([128, TL], F32, tag="tmp", name=f"t1_{kind}{ec}")
                nc.vector.tensor_tensor(t1[:], at[:], src[:], MUL)
                t2 = tmp_pool.tile([128, TL], F32, tag="tmp", name=f"t2_{kind}{ec}")
                nc.vector.tensor_tensor(t2[:], bt[:], swp[:], MUL)
                t3 = tmp_pool.tile([128, TL], F32, tag="tmp", name=f"t3_{kind}{ec}")
                nc.vector.tensor_add(t3[:], t1[:], t2[:])
                nc.vector.tensor_tensor(dst_ap, t3[:], rsb[:], MUL)

            # ---- K: projection + rs + rope, stream to AG input ----
            kT, ssq_k = proj_pass(wk, bk_sb, gk_sb, "k")
            rsb_k = rs_row(ssq_k, "k", fold8=False)
            for ec in range(NEC):
                ks = kst_pool.tile([128, TL], BF16, tag="kst", name=f"ks{ec}")
                rope_chunk(kT[ec], rsb_k, ks[:], ec, "k")
                nc.sync.dma_start(ag_k_in[128 * ec:128 * ec + 128, :], ks[:])

            # ---- AllGather k over the batch group + read-back ----
            nc.gpsimd.collective_compute(
                "AllGather", mybir.AluOpType.bypass, replica_groups=GROUPS,
                ins=[ag_k_in[:].opt()], outs=[ag_k_out[:].opt()],
            )
            for r in range(4):
                nc.sync.dma_start(
                    kf[:].rearrange("p (ec r t) -> p ec r t", ec=NEC, r=4)[:, :, r, :],
                    ag_k_out[D * r:D * r + D, :]
                    .rearrange("(ec p) t -> p ec t", p=128),
                )

            # ---- Q: projection + rs (with 1/sqrt(DH) folded) + rope ----
            qT, ssq_q = proj_pass(wq, bq_sb, gq_sb, "q")
            rsb_q = rs_row(ssq_q, "q", fold8=True)
            for ec in range(NEC):
                rope_chunk(qT[ec], rsb_q, qrot_c[ec][:], ec, "q")

            # ---- full V: t-major projection (replicated, bf16) ----
            for jt in range(NJC):
                ones_cols = vf_c[jt].rearrange("p (h c) -> p h c", c=65)[:, :, 64:65]
                nc.vector.tensor_copy(
                    ones_cols, ones16[:].rearrange("p (f o) -> p f o", o=1)
                )
            wvt = [wv_pool.tile([128, 512], BF16, tag="wv", name=f"wv{i}")
                   for i in range(2 * NDC)]
            for i in range(2 * NDC):
                nc.sync.dma_start(wvt[i][:], wv[:, 512 * i:512 * i + 512])
            for jt in range(NJC):
                xft = xf_pool.tile([128, D], BF16, tag="xf", name=f"xf{jt}")
                nc.sync.dma_start(xft[:], xf[:, D * jt:D * jt + D])
                for eh in range(2):
                    vp = pv.tile([128, 512], F32, tag="pv", name=f"pv{jt}{eh}")
                    for dc in range(NDC):
                        nc.tensor.matmul(
                            vp[:], xft[:, 128 * dc:128 * dc + 128],
                            wvt[2 * dc + eh][:],
                            start=(dc == 0), stop=False,
                        )
                    nc.tensor.matmul(
                        vp[:], ones1b[:], bv_sb[:, 512 * eh:512 * eh + 512],
                        start=False, stop=True,
                    )
                    nc.scalar.activation(
                        vf_c[jt].rearrange("p (h c) -> p h c", c=65)[
                            :, 8 * eh:8 * eh + 8, 0:64],
                        vp[:].rearrange("p (h c) -> p h c", c=64),
                        AF.Identity,
                    )

        # ---------------- phase B: attention ----------------
        with ExitStack() as bctx:
            exp_pool = bctx.enter_context(tc.tile_pool(name="exp", bufs=12))
            dv_pool = bctx.enter_context(tc.tile_pool(name="dv", bufs=4))
            rec_pool = bctx.enter_context(tc.tile_pool(name="rec", bufs=2))
            psc = bctx.enter_context(tc.tile_pool(name="psc", bufs=2, space="PSUM"))
            pav = bctx.enter_context(tc.tile_pool(name="pav", bufs=4, space="PSUM"))

            for hp in range(NEC):
                av = [pav.tile([65, TL], F32, tag="pav", name=f"av{hp}_{i}")
                      for i in range(2)]
                for jc in range(NJC):
                    jsl = slice(128 * jc, 128 * jc + 128)
                    sc = psc.tile([128, 2 * TL], F32, tag="psc", name=f"sc{hp}_{jc}")
                    for hh in range(2):
                        psl = slice(64 * hh, 64 * hh + 64)
                        nc.tensor.matmul(
                            sc[:, TL * hh:TL * hh + TL],
                            kf_c[hp][psl, jsl], qrot_c[hp][psl, :],
                            start=True, stop=True,
                        )
                    ex = exp_pool.tile([128, 2 * TL], BF16, tag="exp",
                                       name=f"ex{hp}_{jc}")
                    nc.scalar.activation(ex[:], sc[:], AF.Exp)
                    for hh in range(2):
                        h4 = 2 * hp + hh
                        nc.tensor.matmul(
                            av[hh][:],
                            vf_c[jc][:, 65 * h4:65 * h4 + 65],
                            ex[:, TL * hh:TL * hh + TL],
                            start=(jc == 0), stop=(jc == NJC - 1),
                        )
                for hh in range(2):
                    avs = dv_pool.tile([65, TL], F32, tag="avs", name=f"avs{hp}{hh}")
                    nc.vector.tensor_copy(avs[:], av[hh][:])
                    rin = rec_pool.tile([1, TL], F32, tag="rin", name=f"rin{hp}{hh}")
                    nc.vector.tensor_copy(rin[:], avs[64:65, :])
                    rec = rec_pool.tile([1, TL], F32, tag="rec", name=f"rec{hp}{hh}")
                    nc.vector.reciprocal_approx_fast(rec[:], rin[:])
                    rcb = rec_pool.tile([64, TL], F32, tag="rcb", name=f"rcb{hp}{hh}")
                    nc.gpsimd.partition_broadcast(rcb[:], rec[0:1, :], channels=64)
                    nc.vector.tensor_tensor(
                        og_c[hp][64 * hh:64 * hh + 64, :], avs[0:64, :], rcb[:], MUL
                    )

        # ---------------- out-projection (fully local) ----------------
        with ExitStack() as octx:
            y_pool = octx.enter_context(tc.tile_pool(name="y", bufs=2))
            py = octx.enter_context(tc.tile_pool(name="py", bufs=2, space="PSUM"))
            for dco in range(NDC):
                yp = py.tile([128, TL], F32, tag="py", name=f"yp{dco}")
                for ec in range(NEC):
                    nc.tensor.matmul(
                        yp[:], wo_sb[:, D * dco + 128 * ec:D * dco + 128 * ec + 128],
                        og_c[ec][:],
                        start=(ec == 0), stop=(ec == NEC - 1),
                    )
                ys = y_pool.tile([128, TL], F32, tag="y", name=f"ys{dco}")
                nc.scalar.activation(ys[:], yp[:], AF.Identity, bias=bo_sb[dco])
                nc.sync.dma_start(yT[128 * dco:128 * dco + 128, :], ys[:])


def build_nc():
    nc = bacc.Bacc("TRN2", target_bir_lowering=False, debug=False,
                   num_devices=N_CORES)
    _emit(nc)
    nc.compile()
    return nc


_NC_CACHE = None


def _get_nc():
    global _NC_CACHE
    if _NC_CACHE is None:
        _NC_CACHE = build_nc()
    return _NC_CACHE


def _host_prep(inputs):
    import ml_dtypes
    bf16 = ml_dtypes.bfloat16

    x = np.ascontiguousarray(np.asarray(inputs["x"], dtype=np.float32))
    pe = np.asarray(inputs["pe"], dtype=np.float32)[0, 0]      # [S, D//2, 2, 2]
    Wq = np.asarray(inputs["Wq"], dtype=np.float32)
    bq = np.asarray(inputs["bq"], dtype=np.float32)
    Wk = np.asarray(inputs["Wk"], dtype=np.float32)
    bk = np.asarray(inputs["bk"], dtype=np.float32)
    Wv = np.asarray(inputs["Wv"], dtype=np.float32)
    bv = np.asarray(inputs["bv"], dtype=np.float32)
    qn = np.asarray(inputs["qn_w"], dtype=np.float32)
    kn = np.asarray(inputs["kn_w"], dtype=np.float32)
    Wo = np.asarray(inputs["Wo"], dtype=np.float32)
    bo = np.asarray(inputs["bo"], dtype=np.float32)

    Wq_f = Wq * qn[:, None]
    bq_f = qn * bq
    Wk_f = Wk * kn[:, None]
    bk_f = kn * bk

    A = np.empty((D, S), np.float32)
    Bm = np.empty((D, S), np.float32)
    A[0::2, :] = pe[:, :, 0, 0].T
    A[1::2, :] = pe[:, :, 1, 1].T
    Bm[0::2, :] = pe[:, :, 0, 1].T
    Bm[1::2, :] = pe[:, :, 1, 0].T

    perm = np.zeros((128, 128), np.float32)
    idx = np.arange(64)
    perm[2 * idx, 2 * idx + 1] = 1.0
    perm[2 * idx + 1, 2 * idx] = 1.0

    def pack_ec(m):
        # m: [D_in, D_out] -> [128, NEC*D]; out[p, ec*D+dc*128+j] =
        # m[dc*128+p, ec*128+j]
        return np.ascontiguousarray(
            m.reshape(NDC, 128, NEC, 128).transpose(1, 2, 0, 3).reshape(128, NEC * D)
        )

    def pack_v(m):
        # m: [D_in, D_out] -> [128, 2*NDC*512]; slice 512*(2*dc+eh) holds
        # the moving block m[dc*128:+128, eh*512:+512]
        return np.ascontiguousarray(
            m.reshape(NDC, 128, 2, 512).transpose(1, 0, 2, 3)
            .reshape(128, 2 * NDC * 512).astype(bf16)
        )

    def pack_o(m):
        # m: [D_e, D_out] -> [128, NDC*D]; out[p, dco*D+ec*128+j] =
        # m[ec*128+p, dco*128+j]
        return np.ascontiguousarray(
            m.reshape(NEC, 128, NDC, 128).transpose(1, 2, 0, 3).reshape(128, NDC * D)
        )

    wq_p = pack_ec(Wq_f.T)
    wk_p = pack_ec(Wk_f.T)
    wv_p = pack_v(Wv.T)
    wo_p = pack_o(Wo.T)

    smalls = np.zeros((128, 26), np.float32)
    smalls[:, 0:8] = bq_f.reshape(8, 128).T
    smalls[:, 8:16] = bk_f.reshape(8, 128).T
    smalls[:, 16:24] = bo.reshape(8, 128).T
    smalls[0, 24] = EPS
    smalls[0, 25] = LN8
    gqk = np.zeros((128, 16), np.float32)
    gqk[:, 0:8] = (1.0 / qn ** 2).reshape(8, 128).T
    gqk[:, 8:16] = (1.0 / kn ** 2).reshape(8, 128).T
    bv_c = np.ascontiguousarray(bv)[None, :].astype(bf16)

    in_maps = []
    for b in range(B):
        xTb = np.ascontiguousarray(x[b].T)  # [D, S]
        # jt-major bf16 copy for the full-v projection
        xf_b = np.ascontiguousarray(
            xTb.reshape(NDC, 128, NJC, 128).transpose(1, 2, 0, 3)
            .reshape(128, NJC * D).astype(bf16)
        )
        for tb in range(4):
            tsl = slice(TL * tb, TL * tb + TL)
            xL_b = np.ascontiguousarray(
                xTb[:, tsl].reshape(NDC, 128, TL).transpose(1, 0, 2)
                .reshape(128, NDC * TL)
            )
            in_maps.append({
                "xL": xL_b,
                "xf": xf_b,
                "wq": wq_p,
                "wk": wk_p,
                "wv": wv_p,
                "wo": wo_p,
                "smalls": smalls,
                "gqk": gqk,
                "bv": bv_c,
                "rope_a": np.ascontiguousarray(A[:, tsl]),
                "rope_b": np.ascontiguousarray(Bm[:, tsl]),
                "perm": perm,
            })
    return in_maps


def kernel(**inputs):
    nc = _get_nc()
    in_maps = _host_prep(inputs)
    res = run_bass_kernel_spmd(
        nc, in_maps, core_ids=list(range(N_CORES)), trace=TRACE
    )
    if TRACE and res.exec_time_ns is not None:
        print(f"HW exec time: {res.exec_time_ns} ns")
    y = np.empty((B, S, D), np.float32)
    for c in range(N_CORES):
        b, tb = c // 4, c % 4
        y[b][TL * tb:TL * tb + TL, :] = res.results[c]["yT"].T
    return y


# revision 13
# speedup vs baseline: 1.1344x; 1.0201x over previous
"""Distributed CrossAttention (self-attention) kernel for 8 TRN2 NeuronCores.

Problem: B=2, S=2048, D=1024, H=16, DH=64, fp32.
  q/k/v = x@W.T + b; RMSNorm(q/k over full D); RoPE; SDPA; out-proj.

Sharding (token-parallel): core c -> (batch b = c//4, token block tb = c%4,
tokens 512*tb .. 512*tb+512). Each core computes q/k at FULL embedding
width for its 512 tokens, so RMSNorm and RoPE are fully local (no
AllReduce). Two half-AllGathers per batch group share the roped/normalized
k (bf16) split by head-halves, so attention on heads 0-7 starts while
heads 8-15 are still in flight; v is cheap to project, so every core
computes the FULL v locally from a bf16 copy of x (replicated flops
instead of a collective), split by e-half and interleaved into the
attention stream so the PE never sits idle. SDPA and the out-projection
run fully locally for the core's 512 queries over all 2048 keys and 16
heads. The program is rank-independent (pure SPMD).

Key engine/bandwidth tricks:
  - All projection operands are bf16 (error budget checked against the
    reference: same 1e-2 as fp32r), and every multi-100KB stream is split
    into ~128KB dma_starts so several DMA queues run in parallel (one
    queue sustains only ~22GB/s).
  - rs = exp(-0.5*ln(ssq/D+eps)) on ScalarE; ln+exp live in the same
    activation table as the softmax exp -> zero table-switch thrash.
    1/sqrt(DH) rides the exp bias; RMSNorm gains are folded into W.
  - The softmax denominator rides the AV matmul as a ones-column of v;
    its reciprocal uses DVE reciprocal_approx_fast, broadcast on gpsimd,
    divide fused into the bf16 og eviction.
  - kf read-backs are emitted after all compute streams so their
    AG-gated waits never block a DMA queue ahead of compute loads.
  - wo is prefetched in bf16 at kernel start; the out-projection is
    local (no collective, no tail stall).
"""
import numpy as np
from contextlib import ExitStack

import concourse.bass as bass
import concourse.mybir as mybir
import concourse.tile as tile
import concourse.bacc as bacc
from concourse.bass_utils import run_bass_kernel_spmd

F32 = mybir.dt.float32
F32R = mybir.dt.float32r
BF16 = mybir.dt.bfloat16
AF = mybir.ActivationFunctionType
MUL = mybir.AluOpType.mult

B, S, D, H, DH = 2, 2048, 1024, 16, 64
EPS = 1e-5
N_CORES = 8
TL = 512            # tokens per core
NEC = D // 128      # 8 e-chunks (head pairs)
NDC = D // 128      # 8 contraction chunks
NJC = S // 128      # 16 key chunks
GROUPS = [[0, 1, 2, 3], [4, 5, 6, 7]]
LN8 = float(np.log(1.0 / np.sqrt(DH)))

TRACE = False       # test.py flips this for profiling


def _emit(nc):
    xL = nc.declare_dram_parameter("xL", [128, NDC * TL], BF16, isOutput=False)
    xf = nc.declare_dram_parameter("xf", [128, NJC * D], BF16, isOutput=False)
    wq = nc.declare_dram_parameter("wq", [128, NEC * D], BF16, isOutput=False)
    wk = nc.declare_dram_parameter("wk", [128, NEC * D], BF16, isOutput=False)
    wv = nc.declare_dram_parameter("wv", [128, 2 * NDC * 512], BF16, isOutput=False)
    wo = nc.declare_dram_parameter("wo", [128, NDC * D], F32, isOutput=False)
    # smalls: cols 0-7 bq, 8-15 bk, 16-23 bo, [0,24] eps, [0,25] ln(1/8)
    smalls = nc.declare_dram_parameter("smalls", [128, 26], F32, isOutput=False)
    gqk = nc.declare_dram_parameter("gqk", [128, 16], F32, isOutput=False)
    bv = nc.declare_dram_parameter("bv", [1, D], BF16, isOutput=False)
    ra = nc.declare_dram_parameter("rope_a", [D, TL], BF16, isOutput=False)
    rb = nc.declare_dram_parameter("rope_b", [D, TL], BF16, isOutput=False)
    pm = nc.declare_dram_parameter("perm", [128, 128], F32, isOutput=False)
    yT = nc.declare_dram_parameter("yT", [D, TL], F32, isOutput=True)

    with tile.TileContext(nc) as tc, ExitStack() as ctx:
        # ---------------- persistent pools ----------------
        kf_pool = ctx.enter_context(tc.tile_pool(name="kf", bufs=1))
        vf_pool = ctx.enter_context(tc.tile_pool(name="vf", bufs=1))
        qrot_pool = ctx.enter_context(tc.tile_pool(name="qrot", bufs=1))
        og_pool = ctx.enter_context(tc.tile_pool(name="og", bufs=1))
        wo_pool = ctx.enter_context(tc.tile_pool(name="wop", bufs=1))
        wv_pool = ctx.enter_context(tc.tile_pool(name="wvp", bufs=1))
        small = ctx.enter_context(tc.tile_pool(name="small", bufs=1))
        dram = ctx.enter_context(tc.tile_pool(name="dram", bufs=1, space="DRAM"))

        # ---------------- constants / small loads ----------------
        sm = small.tile([128, 26], F32, tag="sm", name="sm")
        nc.sync.dma_start(sm[:], smalls[:])
        gq_t = small.tile([128, 16], F32R, tag="gqk", name="gq_t")
        nc.sync.dma_start(gq_t[:], gqk[:].bitcast(F32R))
        bq_sb = [sm[:, ec:ec + 1] for ec in range(NEC)]
        bk_sb = [sm[:, 8 + ec:9 + ec] for ec in range(NEC)]
        bo_sb = [sm[:, 16 + ec:17 + ec] for ec in range(NEC)]
        eps_t = sm[0:1, 24:25]
        ln8_t = sm[0:1, 25:26]
        gq_sb = [gq_t[:, ec:ec + 1] for ec in range(NEC)]
        gk_sb = [gq_t[:, 8 + ec:9 + ec] for ec in range(NEC)]
        bv_sb = small.tile([1, D], BF16, tag="bvrow")
        nc.sync.dma_start(bv_sb[:], bv[:])
        ones1b = small.tile([1, 128], BF16, tag="ones1b")
        nc.vector.memset(ones1b[:], 1.0)
        ones16 = small.tile([128, 16], F32, tag="ones16")
        nc.vector.memset(ones16[:], 1.0)
        perm_sb = small.tile([128, 128], F32R, tag="perm")
        nc.sync.dma_start(perm_sb[:], pm[:].bitcast(F32R))

        # persistent activation storage
        kf = kf_pool.tile([128, NEC * S], BF16, tag="kf", name="kf", bufs=1)
        kf_c = [kf[:, S * ec:S * ec + S] for ec in range(NEC)]
        vf = vf_pool.tile([128, NJC * H * 65], BF16, tag="vf", name="vf", bufs=1)
        vf_c = [vf[:, H * 65 * jt:H * 65 * (jt + 1)] for jt in range(NJC)]
        qrot = qrot_pool.tile([128, NEC * TL], BF16, tag="qrot", name="qrot", bufs=1)
        qrot_c = [qrot[:, TL * ec:TL * ec + TL] for ec in range(NEC)]
        og = og_pool.tile([128, NEC * TL], BF16, tag="og", name="og", bufs=1)
        og_c = [og[:, TL * ec:TL * ec + TL] for ec in range(NEC)]

        # x for Q/K (bf16, loaded once)
        x_pool = ctx.enter_context(tc.tile_pool(name="xp", bufs=1))
        xt = [x_pool.tile([128, TL], BF16, tag="xt", name=f"xt{i}", bufs=8)
              for i in range(NDC)]
        for dc in range(NDC):
            nc.sync.dma_start(xt[dc][:], xL[:, TL * dc:TL * dc + TL])

        # wv slices (prefetched, resident)
        wvt = [wv_pool.tile([128, 512], BF16, tag="wv", name=f"wv{i}", bufs=16)
               for i in range(2 * NDC)]
        for i in range(2 * NDC):
            nc.sync.dma_start(wvt[i][:], wv[:, 512 * i:512 * i + 512])

        # wo prefetch (bf16 convert on the gpsimd SWDGE queue)
        wo_sb = wo_pool.tile([128, NDC * D], BF16, tag="wo", name="wo_sb", bufs=1)
        for dco in range(NDC):
            nc.gpsimd.dma_start(
                wo_sb[:, D * dco:D * dco + D], wo[:, D * dco:D * dco + D]
            )

        # AllGather DRAM buffers (k split by head-halves)
        ag_in = [dram.tile([D // 2, TL], BF16, tag=f"agi{i}", name=f"ag_in{i}")
                 for i in range(2)]
        ag_out = [dram.tile([2 * D, TL], BF16, tag=f"ago{i}", name=f"ag_out{i}")
                  for i in range(2)]

        # ---------------- phase A ----------------
        with ExitStack() as actx:
            xf_pool = actx.enter_context(tc.tile_pool(name="xfp", bufs=3))
            pv = actx.enter_context(tc.tile_pool(name="pv", bufs=1, space="PSUM"))
            pctx = ExitStack()
            w_pool = pctx.enter_context(tc.tile_pool(name="wp", bufs=3))
            qk_pool = pctx.enter_context(tc.tile_pool(name="qk", bufs=8))
            kst_pool = pctx.enter_context(tc.tile_pool(name="kst", bufs=2))
            sq_pool = pctx.enter_context(tc.tile_pool(name="sq", bufs=2))
            ab_pool = pctx.enter_context(tc.tile_pool(name="ab", bufs=4))
            tmp_pool = pctx.enter_context(tc.tile_pool(name="tmp", bufs=6))
            rs_pool = pctx.enter_context(tc.tile_pool(name="rs", bufs=3))
            pproj = pctx.enter_context(tc.tile_pool(name="pproj", bufs=2, space="PSUM"))
            pssq = pctx.enter_context(tc.tile_pool(name="pssq", bufs=1, space="PSUM"))
            psw = pctx.enter_context(tc.tile_pool(name="psw", bufs=2, space="PSUM"))

            def proj_pass(wparam, bias_sb, g_sb, kind):
                dst = []
                ssq_ps = pssq.tile([1, TL], F32, tag="pssq", name=f"ssq_{kind}")
                for ec in range(NEC):
                    wsl = w_pool.tile([128, D], BF16, tag="w", name=f"w_{kind}{ec}")
                    for q4 in range(4):
                        csl = slice(D * ec + 256 * q4, D * ec + 256 * q4 + 256)
                        nc.sync.dma_start(wsl[:, 256 * q4:256 * q4 + 256],
                                          wparam[:, csl])
                    qp = pproj.tile([128, TL], F32, tag="pproj", name=f"p_{kind}{ec}")
                    for dc in range(NDC):
                        nc.tensor.matmul(
                            qp[:], wsl[:, 128 * dc:128 * dc + 128], xt[dc][:],
                            start=(dc == 0), stop=(dc == NDC - 1),
                        )
                    d = qk_pool.tile([128, TL], F32R, tag="qk", name=f"{kind}T{ec}")
                    nc.scalar.activation(d[:], qp[:], AF.Identity, bias=bias_sb[ec])
                    dst.append(d)
                    sq = sq_pool.tile([128, TL], F32R, tag="sq", name=f"sq_{kind}{ec}")
                    nc.scalar.activation(sq[:], qp[:], AF.Square, bias=bias_sb[ec])
                    nc.tensor.matmul(
                        ssq_ps[:], g_sb[ec], sq[:],
                        start=(ec == 0), stop=(ec == NEC - 1),
                    )
                ssq_c = rs_pool.tile([1, TL], F32, tag="rs", name=f"ssqc_{kind}")
                nc.vector.tensor_copy(ssq_c[:], ssq_ps[:])
                return dst, ssq_c

            def rs_row(ssq_c, kind, fold8):
                lnv = rs_pool.tile([1, TL], F32, tag="rs", name=f"ln_{kind}")
                nc.scalar.activation(lnv[:], ssq_c[:], AF.Ln, scale=1.0 / D,
                                     bias=eps_t)
                rsr = rs_pool.tile([1, TL], F32, tag="rs", name=f"rsr_{kind}")
                if fold8:
                    nc.scalar.activation(rsr[:], lnv[:], AF.Exp, scale=-0.5,
                                         bias=ln8_t)
                else:
                    nc.scalar.activation(rsr[:], lnv[:], AF.Exp, scale=-0.5)
                rsb = rs_pool.tile([128, TL], F32, tag="rsb", name=f"rsb_{kind}",
                                   bufs=2)
                nc.gpsimd.partition_broadcast(rsb[:], rsr[0:1, :], channels=128)
                return rsb

            def rope_chunk(src, rsb, dst_ap, ec, kind):
                esl = slice(128 * ec, 128 * ec + 128)
                at = ab_pool.tile([128, TL], BF16, tag="ab", name=f"at_{kind}{ec}")
                bt = ab_pool.tile([128, TL], BF16, tag="ab", name=f"bt_{kind}{ec}")
                nc.sync.dma_start(at[:], ra[esl, :])
                nc.sync.dma_start(bt[:], rb[esl, :])
                swp = psw.tile([128, TL], F32, tag="psw", name=f"sw_{kind}{ec}")
                nc.tensor.matmul(swp[:], perm_sb[:], src[:], start=True, stop=True)
                t1 = tmp_pool.tile([128, TL], F32, tag="tmp", name=f"t1_{kind}{ec}")
                nc.vector.tensor_tensor(t1[:], at[:], src[:], MUL)
                t2 = tmp_pool.tile([128, TL], F32, tag="tmp", name=f"t2_{kind}{ec}")
                nc.vector.tensor_tensor(t2[:], bt[:], swp[:], MUL)
                t3 = tmp_pool.tile([128, TL], F32, tag="tmp", name=f"t3_{kind}{ec}")
                nc.vector.tensor_add(t3[:], t1[:], t2[:])
                nc.vector.tensor_tensor(dst_ap, t3[:], rsb[:], MUL)

            # ---- K: projection + rs + rope, stream to the two AG inputs ----
            kT, ssq_k = proj_pass(wk, bk_sb, gk_sb, "k")
            rsb_k = rs_row(ssq_k, "k", fold8=False)
            for half in range(2):
                for e2 in range(4):
                    ec = 4 * half + e2
                    ks = kst_pool.tile([128, TL], BF16, tag="kst", name=f"ks{ec}")
                    rope_chunk(kT[ec], rsb_k, ks[:], ec, "k")
                    nc.sync.dma_start(
                        ag_in[half][128 * e2:128 * e2 + 128, :], ks[:]
                    )
                nc.gpsimd.collective_compute(
                    "AllGather", mybir.AluOpType.bypass, replica_groups=GROUPS,
                    ins=[ag_in[half][:].opt()], outs=[ag_out[half][:].opt()],
                )

            # ---- Q: projection + rs (with 1/sqrt(DH) folded) + rope ----
            qT, ssq_q = proj_pass(wq, bq_sb, gq_sb, "q")
            rsb_q = rs_row(ssq_q, "q", fold8=True)
            for ec in range(NEC):
                rope_chunk(qT[ec], rsb_q, qrot_c[ec][:], ec, "q")
            pctx.close()

            # ---- full V (replicated, bf16), emitted per e-half ----
            for jt in range(NJC):
                ones_cols = vf_c[jt].rearrange("p (h c) -> p h c", c=65)[:, :, 64:65]
                nc.vector.tensor_copy(
                    ones_cols, ones16[:].rearrange("p (f o) -> p f o", o=1)
                )

            def emit_v(jts, eh):
                for jt in jts:
                    xft = xf_pool.tile([128, D], BF16, tag="xf",
                                       name=f"xf{jt}_{eh}")
                    for hh in range(2):
                        nc.sync.dma_start(
                            xft[:, 512 * hh:512 * hh + 512],
                            xf[:, D * jt + 512 * hh:D * jt + 512 * hh + 512],
                        )
                    vp = pv.tile([128, 512], F32, tag="pv", name=f"pv{jt}{eh}")
                    for dc in range(NDC):
                        nc.tensor.matmul(
                            vp[:], xft[:, 128 * dc:128 * dc + 128],
                            wvt[2 * dc + eh][:],
                            start=(dc == 0), stop=False,
                        )
                    nc.tensor.matmul(
                        vp[:], ones1b[:], bv_sb[:, 512 * eh:512 * eh + 512],
                        start=False, stop=True,
                    )
                    nc.scalar.activation(
                        vf_c[jt].rearrange("p (h c) -> p h c", c=65)[
                            :, 8 * eh:8 * eh + 8, 0:64],
                        vp[:].rearrange("p (h c) -> p h c", c=64),
                        AF.Identity,
                    )

            # xf tiles must stay live across both e-halves
            emit_v(list(range(NJC)), 0)

            def readback(half):
                for r in range(4):
                    for e2 in range(2):
                        nc.sync.dma_start(
                            kf[:].rearrange("p (ec r t) -> p ec r t",
                                            ec=NEC, r=4)[
                                :, 4 * half + 2 * e2:4 * half + 2 * e2 + 2, r, :],
                            ag_out[half][
                                512 * r + 256 * e2:512 * r + 256 * e2 + 256, :]
                            .rearrange("(ec p) t -> p ec t", p=128),
                        )

            readback(0)
            readback(1)

            # ---------------- phase B: attention ----------------
            with ExitStack() as bctx:
                exp_pool = bctx.enter_context(tc.tile_pool(name="exp", bufs=12))
                dv_pool = bctx.enter_context(tc.tile_pool(name="dv", bufs=4))
                rec_pool = bctx.enter_context(tc.tile_pool(name="rec", bufs=2))
                psc = bctx.enter_context(tc.tile_pool(name="psc", bufs=2,
                                                      space="PSUM"))
                pav = bctx.enter_context(tc.tile_pool(name="pav", bufs=3,
                                                      space="PSUM"))

                def attn_hp(hp):
                    av = [pav.tile([65, TL], F32, tag="pav", name=f"av{hp}_{i}")
                          for i in range(2)]
                    for jc in range(NJC):
                        jsl = slice(128 * jc, 128 * jc + 128)
                        sc = psc.tile([128, 2 * TL], F32, tag="psc",
                                      name=f"sc{hp}_{jc}")
                        for hh in range(2):
                            psl = slice(64 * hh, 64 * hh + 64)
                            nc.tensor.matmul(
                                sc[:, TL * hh:TL * hh + TL],
                                kf_c[hp][psl, jsl], qrot_c[hp][psl, :],
                                start=True, stop=True,
                            )
                        ex = exp_pool.tile([128, 2 * TL], BF16, tag="exp",
                                           name=f"ex{hp}_{jc}")
                        nc.scalar.activation(ex[:], sc[:], AF.Exp)
                        for hh in range(2):
                            h4 = 2 * hp + hh
                            nc.tensor.matmul(
                                av[hh][:],
                                vf_c[jc][:, 65 * h4:65 * h4 + 65],
                                ex[:, TL * hh:TL * hh + TL],
                                start=(jc == 0), stop=(jc == NJC - 1),
                            )
                    for hh in range(2):
                        avs = dv_pool.tile([65, TL], F32, tag="avs",
                                           name=f"avs{hp}{hh}")
                        nc.vector.tensor_copy(avs[:], av[hh][:])
                        rin = rec_pool.tile([1, TL], F32, tag="rin",
                                            name=f"rin{hp}{hh}")
                        nc.vector.tensor_copy(rin[:], avs[64:65, :])
                        rec = rec_pool.tile([1, TL], F32, tag="rec",
                                            name=f"rec{hp}{hh}")
                        nc.vector.reciprocal_approx_fast(rec[:], rin[:])
                        rcb = rec_pool.tile([64, TL], F32, tag="rcb",
                                            name=f"rcb{hp}{hh}")
                        nc.gpsimd.partition_broadcast(rcb[:], rec[0:1, :],
                                                      channels=64)
                        nc.vector.tensor_tensor(
                            og_c[hp][64 * hh:64 * hh + 64, :], avs[0:64, :],
                            rcb[:], MUL
                        )

                attn_hp(0)
                attn_hp(1)
                emit_v(list(range(0, 8)), 1)
                attn_hp(2)
                emit_v(list(range(8, NJC)), 1)
                attn_hp(3)
                for hp in range(4, NEC):
                    attn_hp(hp)

        # ---------------- out-projection (fully local) ----------------
        with ExitStack() as octx:
            y_pool = octx.enter_context(tc.tile_pool(name="y", bufs=2))
            py = octx.enter_context(tc.tile_pool(name="py", bufs=2, space="PSUM"))
            for dco in range(NDC):
                yp = py.tile([128, TL], F32, tag="py", name=f"yp{dco}")
                for ec in range(NEC):
                    nc.tensor.matmul(
                        yp[:], wo_sb[:, D * dco + 128 * ec:D * dco + 128 * ec + 128],
                        og_c[ec][:],
                        start=(ec == 0), stop=(ec == NEC - 1),
                    )
                ys = y_pool.tile([128, TL], F32, tag="y", name=f"ys{dco}")
                nc.scalar.activation(ys[:], yp[:], AF.Identity, bias=bo_sb[dco])
                nc.sync.dma_start(yT[128 * dco:128 * dco + 128, :], ys[:])


def build_nc():
    nc = bacc.Bacc("TRN2", target_bir_lowering=False, debug=False,
                   num_devices=N_CORES)
    _emit(nc)
    nc.compile()
    return nc


_NC_CACHE = None


def _get_nc():
    global _NC_CACHE
    if _NC_CACHE is None:
        _NC_CACHE = build_nc()
    return _NC_CACHE


def _host_prep(inputs):
    import ml_dtypes
    bf16 = ml_dtypes.bfloat16

    x = np.ascontiguousarray(np.asarray(inputs["x"], dtype=np.float32))
    pe = np.asarray(inputs["pe"], dtype=np.float32)[0, 0]      # [S, D//2, 2, 2]
    Wq = np.asarray(inputs["Wq"], dtype=np.float32)
    bq = np.asarray(inputs["bq"], dtype=np.float32)
    Wk = np.asarray(inputs["Wk"], dtype=np.float32)
    bk = np.asarray(inputs["bk"], dtype=np.float32)
    Wv = np.asarray(inputs["Wv"], dtype=np.float32)
    bv = np.asarray(inputs["bv"], dtype=np.float32)
    qn = np.asarray(inputs["qn_w"], dtype=np.float32)
    kn = np.asarray(inputs["kn_w"], dtype=np.float32)
    Wo = np.asarray(inputs["Wo"], dtype=np.float32)
    bo = np.asarray(inputs["bo"], dtype=np.float32)

    Wq_f = Wq * qn[:, None]
    bq_f = qn * bq
    Wk_f = Wk * kn[:, None]
    bk_f = kn * bk

    A = np.empty((D, S), np.float32)
    Bm = np.empty((D, S), np.float32)
    A[0::2, :] = pe[:, :, 0, 0].T
    A[1::2, :] = pe[:, :, 1, 1].T
    Bm[0::2, :] = pe[:, :, 0, 1].T
    Bm[1::2, :] = pe[:, :, 1, 0].T

    perm = np.zeros((128, 128), np.float32)
    idx = np.arange(64)
    perm[2 * idx, 2 * idx + 1] = 1.0
    perm[2 * idx + 1, 2 * idx] = 1.0

    def pack_ec(m):
        # m: [D_in, D_out] -> [128, NEC*D]; out[p, ec*D+dc*128+j] =
        # m[dc*128+p, ec*128+j]
        return np.ascontiguousarray(
            m.reshape(NDC, 128, NEC, 128).transpose(1, 2, 0, 3)
            .reshape(128, NEC * D).astype(bf16)
        )

    def pack_v(m):
        # m: [D_in, D_out] -> [128, 2*NDC*512]; slice 512*(2*dc+eh) holds
        # the moving block m[dc*128:+128, eh*512:+512]
        return np.ascontiguousarray(
            m.reshape(NDC, 128, 2, 512).transpose(1, 0, 2, 3)
            .reshape(128, 2 * NDC * 512).astype(bf16)
        )

    def pack_o(m):
        # m: [D_e, D_out] -> [128, NDC*D]; out[p, dco*D+ec*128+j] =
        # m[ec*128+p, dco*128+j]
        return np.ascontiguousarray(
            m.reshape(NEC, 128, NDC, 128).transpose(1, 2, 0, 3).reshape(128, NDC * D)
        )

    wq_p = pack_ec(Wq_f.T)
    wk_p = pack_ec(Wk_f.T)
    wv_p = pack_v(Wv.T)
    wo_p = pack_o(Wo.T)

    smalls = np.zeros((128, 26), np.float32)
    smalls[:, 0:8] = bq_f.reshape(8, 128).T
    smalls[:, 8:16] = bk_f.reshape(8, 128).T
    smalls[:, 16:24] = bo.reshape(8, 128).T
    smalls[0, 24] = EPS
    smalls[0, 25] = LN8
    gqk = np.zeros((128, 16), np.float32)
    gqk[:, 0:8] = (1.0 / qn ** 2).reshape(8, 128).T
    gqk[:, 8:16] = (1.0 / kn ** 2).reshape(8, 128).T
    bv_c = np.ascontiguousarray(bv)[None, :].astype(bf16)

    in_maps = []
    for b in range(B):
        xTb = np.ascontiguousarray(x[b].T)  # [D, S]
        xf_b = np.ascontiguousarray(
            xTb.reshape(NDC, 128, NJC, 128).transpose(1, 2, 0, 3)
            .reshape(128, NJC * D).astype(bf16)
        )
        for tb in range(4):
            tsl = slice(TL * tb, TL * tb + TL)
            xL_b = np.ascontiguousarray(
                xTb[:, tsl].reshape(NDC, 128, TL).transpose(1, 0, 2)
                .reshape(128, NDC * TL).astype(bf16)
            )
            in_maps.append({
                "xL": xL_b,
                "xf": xf_b,
                "wq": wq_p,
                "wk": wk_p,
                "wv": wv_p,
                "wo": wo_p,
                "smalls": smalls,
                "gqk": gqk,
                "bv": bv_c,
                "rope_a": np.ascontiguousarray(A[:, tsl]).astype(bf16),
                "rope_b": np.ascontiguousarray(Bm[:, tsl]).astype(bf16),
                "perm": perm,
            })
    return in_maps


def kernel(**inputs):
    nc = _get_nc()
    in_maps = _host_prep(inputs)
    res = run_bass_kernel_spmd(
        nc, in_maps, core_ids=list(range(N_CORES)), trace=TRACE
    )
    if TRACE and res.exec_time_ns is not None:
        print(f"HW exec time: {res.exec_time_ns} ns")
    y = np.empty((B, S, D), np.float32)
    for c in range(N_CORES):
        b, tb = c // 4, c % 4
        y[b][TL * tb:TL * tb + TL, :] = res.results[c]["yT"].T
    return y
